# revision 1
# baseline (speedup 1.0000x reference)
"""GNN message-passing kernel for Trainium2 (8 NeuronCores, batch-sharded).

Computes, for each batch b:
    neigh[i, d] = max_j (A[b, j, i] * x[b, j, d])      (== reference masked max)
    out = x @ W_self.T + neigh @ W_neigh.T

Algorithm per batch (exact on {0,1} adjacency with at least one non-neighbor
per node, which the reference's where(...,0).max() semantics imply):
  - x^T and A^T built with PE transposes (identity matmul).
  - A^T mapped to additive penalties BIG*(A^T - 1) in {0, -BIG}, flattened
    into one SBUF partition.
  - Per group of 6 targets (two PSUM banks, 3 slots each): PE assembles
    x^T + penalty_i in PSUM (transpose-matmul x-fills + K=1 ones-matmul
    penalty broadcast, bf16 where exact), then one grouped 4D-AP DVE
    tensor_reduce computes max over j for all 6 targets in a single op.
  - neigh = relu(that max); final projections as two PSUM-accumulated matmuls.
"""

import numpy as np

import concourse.bacc as bacc
import concourse.bass as bass
import concourse.mybir as mybir
import concourse.tile as tile
from concourse.bass_utils import run_bass_kernel_spmd
from concourse.masks import make_identity

B, S, D = 32, 150, 128
NCORES = 8
BPC = B // NCORES  # batches per core
BIG = 1024.0  # penalty scale; |x| < 6 so 1024 dominates and stays exact in f32
GRP = 6  # targets per PSUM tile (two banks; 3 slots of 150 per 512-wide bank)
BANK = 512  # fp32 elements per PSUM bank partition

f32 = mybir.dt.float32
bf16 = mybir.dt.bfloat16
i32 = mybir.dt.int32

_PROGRAM_CACHE: dict[str, bass.Bass] = {}


def _build_batch(nc, tc, cpool, wpool, ppool, mbpool, consts, x_d, a_d, out_d, b):
    ident, ident_bf, ones1, wst_sb, wnt_sb = consts

    # ---- load x (2 j-chunks)
    x0 = wpool.tile([128, D], f32, tag="x0")
    x1 = wpool.tile([22, D], f32, tag="x1")
    nc.sync.dma_start(x0[:], x_d[b, 0:128, :])
    nc.sync.dma_start(x1[:], x_d[b, 128:150, :])

    # ---- xT = x^T [D, S] via PE transpose
    xT_ps = ppool.tile([D, S], f32, tag="tps")
    nc.tensor.transpose(xT_ps[:, 0:128], x0[:], ident[:])
    nc.tensor.transpose(xT_ps[:, 128:150], x1[:], ident[0:22, 0:22])
    xT = wpool.tile([D, S], f32, tag="xT_sb")
    nc.scalar.copy(xT[:], xT_ps[:])

    # ---- load A int32 (2 j-chunks), cast to bf16 on DVE ({0,1}: exact)
    a0_i = wpool.tile([128, S], i32, tag="a0i")
    a1_i = wpool.tile([22, S], i32, tag="a1i")
    nc.sync.dma_start(a0_i[:], a_d[b, 0:128, :])
    nc.sync.dma_start(a1_i[:], a_d[b, 128:150, :])
    a0 = wpool.tile([128, S], bf16, tag="a0")
    a1 = wpool.tile([22, S], bf16, tag="a1")
    nc.vector.tensor_copy(a0[:], a0_i[:])
    nc.vector.tensor_copy(a1[:], a1_i[:])

    # ---- A^T via 4 bf16 PE transposes, then penalty BIG*(A^T - 1) in bf16
    at0_ps = ppool.tile([128, S], bf16, tag="tps")
    nc.tensor.transpose(at0_ps[:, 0:128], a0[:, 0:128], ident_bf[:])
    nc.tensor.transpose(at0_ps[:, 128:150], a1[:, 0:128], ident_bf[0:22, 0:22])
    pen0 = wpool.tile([128, S], bf16, tag="pen0")
    nc.scalar.activation(
        pen0[:], at0_ps[:], mybir.ActivationFunctionType.Copy, bias=-BIG, scale=BIG
    )
    at1_ps = ppool.tile([22, S], bf16, tag="tps")
    nc.tensor.transpose(at1_ps[:, 0:128], a0[:, 128:150], ident_bf[:])
    nc.tensor.transpose(at1_ps[:, 128:150], a1[:, 128:150], ident_bf[0:22, 0:22])
    pen1 = wpool.tile([22, S], bf16, tag="pen1")
    nc.scalar.activation(
        pen1[:], at1_ps[:], mybir.ActivationFunctionType.Copy, bias=-BIG, scale=BIG
    )

    # ---- flatten penalties into one partition: pflat[0, i*S + j]  (bf16)
    pflat = wpool.tile([1, S * S], bf16, tag="pflat")
    nc.sync.dma_start(pflat[0:1, 0 : 128 * S], pen0[:, :])
    nc.sync.dma_start(pflat[0:1, 128 * S : S * S], pen1[:, :])

    # ---- masked max per group of GRP targets: reduce_max_j (xT + penalty_i)
    # Multi-bank PSUM tile; HALF slots of S columns per 512-wide bank.
    HALF = BANK // S
    NBANK = GRP // HALF
    rT = wpool.tile([D, S], f32, tag="rT")
    for i0 in range(0, S, GRP):
        g = min(GRP, S - i0)
        nbank = (g + HALF - 1) // HALF
        mb = mbpool.tile([D, NBANK * BANK], f32, tag="mb")
        # penalty broadcast opens each bank's accumulation group
        for nb in range(nbank):
            lo_i = i0 + nb * HALF
            hi_i = min(i0 + (nb + 1) * HALF, i0 + g)
            nc.tensor.matmul(
                mb[:, nb * BANK : nb * BANK + (hi_i - lo_i) * S],
                ones1[:],
                pflat[0:1, lo_i * S : hi_i * S],
                start=True,
                stop=False,
            )
        # x-fill: transpose-matmuls accumulate x^T into each slot
        for c in range(g):
            base = (c // HALF) * BANK + (c % HALF) * S
            last = c % HALF == HALF - 1 or c == g - 1  # closes this bank
            nc.tensor.matmul(
                mb[:, base : base + 128],
                x0[:],
                ident[:],
                is_transpose=True,
                start=False,
                stop=False,
            )
            nc.tensor.matmul(
                mb[:, base + 128 : base + 150],
                x1[:],
                ident[0:22, 0:22],
                is_transpose=True,
                start=False,
                stop=last,
            )
        if g == GRP:
            red_in = (
                mb[:]
                .rearrange("p (b r) -> p b r", b=NBANK)[:, :, 0 : HALF * S]
                .rearrange("p b (g s) -> p b g s", g=HALF)
            )
            nc.vector.tensor_reduce(
                out=rT[:, i0 : i0 + GRP],
                in_=red_in,
                axis=mybir.AxisListType.X,
                op=mybir.AluOpType.max,
            )
        else:
            for nb in range(nbank):
                lo_i = i0 + nb * HALF
                hi_i = min(i0 + (nb + 1) * HALF, i0 + g)
                red_in = mb[:, nb * BANK : nb * BANK + (hi_i - lo_i) * S].rearrange(
                    "p (g s) -> p g s", g=hi_i - lo_i
                )
                nc.vector.tensor_reduce(
                    out=rT[:, lo_i:hi_i],
                    in_=red_in,
                    axis=mybir.AxisListType.X,
                    op=mybir.AluOpType.max,
                )

    # ---- neigh^T = relu(rT)
    rT_relu = wpool.tile([D, S], f32, tag="rTrelu")
    nc.scalar.activation(rT_relu[:], rT[:], mybir.ActivationFunctionType.Relu)

    # ---- out = x @ Ws^T + neigh @ Wn^T   (contract d; out [s-chunk, e])
    for c, (lo, hi) in enumerate([(0, 128), (128, 150)]):
        m = hi - lo
        o_ps = ppool.tile([m, D], f32, tag="wtops")
        nc.tensor.matmul(o_ps[:], xT[:, lo:hi], wst_sb[:], start=True, stop=False)
        nc.tensor.matmul(o_ps[:], rT_relu[:, lo:hi], wnt_sb[:], start=False, stop=True)
        o_sb = wpool.tile([m, D], f32, tag=f"osb{c}")
        nc.scalar.copy(o_sb[:], o_ps[:])
        nc.sync.dma_start(out_d[b, lo:hi, :], o_sb[:])


def _build_program() -> bass.Bass:
    if "nc" in _PROGRAM_CACHE:
        return _PROGRAM_CACHE["nc"]

    nc = bacc.Bacc("TRN2", target_bir_lowering=False, debug=False)
    x_d = nc.dram_tensor("x", [BPC, S, D], f32, kind="ExternalInput").ap()
    a_d = nc.dram_tensor("A", [BPC, S, S], i32, kind="ExternalInput").ap()
    ws_d = nc.dram_tensor("ws", [D, D], f32, kind="ExternalInput").ap()
    wn_d = nc.dram_tensor("wn", [D, D], f32, kind="ExternalInput").ap()
    out_d = nc.dram_tensor("out", [BPC, S, D], f32, kind="ExternalOutput").ap()

    with tile.TileContext(nc) as tc:
        with (
            tc.tile_pool(name="const", bufs=1) as cpool,
            tc.tile_pool(name="work", bufs=3) as wpool,
            tc.tile_pool(name="psum", bufs=1, space="PSUM") as ppool,
            tc.tile_pool(name="psum_mb", bufs=3, space="PSUM") as mbpool,
        ):
            ident = cpool.tile([128, 128], f32)
            make_identity(nc, ident[:])
            ident_bf = cpool.tile([128, 128], bf16, tag="identbf")
            nc.vector.tensor_copy(ident_bf[:], ident[:])
            ones1 = cpool.tile([1, 128], bf16, tag="ones1")
            nc.gpsimd.memset(ones1[:], 1.0)

            ws_sb = cpool.tile([D, D], f32, tag="ws")
            wn_sb = cpool.tile([D, D], f32, tag="wn")
            nc.sync.dma_start(ws_sb[:], ws_d[:, :])
            nc.sync.dma_start(wn_sb[:], wn_d[:, :])
            wst_sb = cpool.tile([D, D], f32, tag="wst")
            wnt_sb = cpool.tile([D, D], f32, tag="wnt")
            wt_ps = ppool.tile([D, D], f32, tag="wtops")
            nc.tensor.transpose(wt_ps[:], ws_sb[:], ident[:])
            nc.scalar.copy(wst_sb[:], wt_ps[:])
            wt_ps2 = ppool.tile([D, D], f32, tag="wtops")
            nc.tensor.transpose(wt_ps2[:], wn_sb[:], ident[:])
            nc.scalar.copy(wnt_sb[:], wt_ps2[:])

            consts = (ident, ident_bf, ones1, wst_sb, wnt_sb)
            for b in range(BPC):
                _build_batch(
                    nc, tc, cpool, wpool, ppool, mbpool, consts, x_d, a_d, out_d, b
                )

    nc.compile()
    _PROGRAM_CACHE["nc"] = nc
    return nc


def kernel(x, A, W_self, W_neigh, **kwargs):
    x = np.ascontiguousarray(np.asarray(x, dtype=np.float32))
    A = np.ascontiguousarray(np.asarray(A, dtype=np.int32))
    W_self = np.ascontiguousarray(np.asarray(W_self, dtype=np.float32))
    W_neigh = np.ascontiguousarray(np.asarray(W_neigh, dtype=np.float32))

    nc = _build_program()
    in_maps = [
        {
            "x": x[c * BPC : (c + 1) * BPC],
            "A": A[c * BPC : (c + 1) * BPC],
            "ws": W_self,
            "wn": W_neigh,
        }
        for c in range(NCORES)
    ]
    res = run_bass_kernel_spmd(nc, in_maps, core_ids=list(range(NCORES)), **kwargs)
    out = np.concatenate([res.results[c]["out"] for c in range(NCORES)], axis=0)
    return np.ascontiguousarray(out.astype(np.float32))



# revision 10
# speedup vs baseline: 2.2664x; 2.2664x over previous
"""GNN message-passing kernel for Trainium2 (8 NeuronCores, batch-sharded).

Computes, for each batch b:
    neigh[i, d] = max(0, max_{j: A[b, j, i] == 1} x[b, j, d])
    out = x @ W_self.T + neigh @ W_neigh.T

Subset-table algorithm (exact on {0,1} adjacency; bf16 data path):
  - Split the 150 source nodes j into 30 chunks of C=5. For each chunk,
    build the full 32-entry subset-max table M_g[s, d] = max(0, x[j_k] for
    k in s) with a log-depth lattice sweep on DVE, held in x^T layout
    (d on partitions, subsets on the free axis, stride-0 broadcast of x).
  - PE-transpose quads of chunk tables into [subset, d] layout (4 chunks
    x 32 subsets = 128 rows per transpose).
  - Patterns Pat[g, i] = sum_k 2^k A[5g+k, i] via one small PE matmul;
    broadcast each Pat row over 32 subset rows with constant selector
    matmuls; one DVE is_equal against a per-partition iota turns that
    into one-hot columns.
  - One 32-contraction matmul per chunk selects M_g[Pat[g, i], :] for all
    i at once -> per-chunk masked-max planes [d, i] in PSUM.
  - A pairwise max tree over the 30 planes (split across GpSimd and DVE)
    folds them into neigh^T; projections are two accumulated bf16 matmuls.
"""

import numpy as np
import ml_dtypes

import concourse.bacc as bacc
import concourse.bass as bass
import concourse.mybir as mybir
import concourse.tile as tile
from concourse.bass_utils import run_bass_kernel_spmd

B, S, D = 32, 150, 128
NCORES = 8
BPC = B // NCORES  # batches per core
C = 5              # chunk size (nodes per chunk)
G = S // C         # 30 chunks
NS = 1 << C        # 32 subsets per chunk
NQ = G // 3        # 10 groups of 3 chunks (96 transpose rows; matmul bases 0/32/64)

f32 = mybir.dt.float32
bf16 = mybir.dt.bfloat16
i32 = mybir.dt.int32

_PROGRAM_CACHE: dict[str, bass.Bass] = {}


def _consts():
    p5 = np.zeros((S, G), dtype=np.float32)
    for j in range(S):
        p5[j, j // C] = float(1 << (j % C))
    rall = np.zeros((G, NQ * 128), dtype=np.float32)
    for g in range(G):
        q, c = g // 3, g % 3
        rall[g, q * 128 + c * 32:q * 128 + c * 32 + 32] = 1.0
    iota = (np.arange(128) % NS).astype(np.float32).reshape(128, 1)
    ident = np.eye(128, dtype=np.float32)
    bf = ml_dtypes.bfloat16
    return p5.astype(bf), rall.astype(bf), iota, ident.astype(bf)


def _build_program() -> bass.Bass:
    if "nc" in _PROGRAM_CACHE:
        return _PROGRAM_CACHE["nc"]

    nc = bacc.Bacc("TRN2", target_bir_lowering=False, debug=False)
    x_d = nc.dram_tensor("x", [BPC, S, D], f32, kind="ExternalInput").ap()
    a_d = nc.dram_tensor("A", [BPC, S, S], i32, kind="ExternalInput").ap()
    ws_d = nc.dram_tensor("ws", [D, D], f32, kind="ExternalInput").ap()
    wn_d = nc.dram_tensor("wn", [D, D], f32, kind="ExternalInput").ap()
    p5_d = nc.dram_tensor("p5", [S, G], bf16, kind="ExternalInput").ap()
    rall_d = nc.dram_tensor("rall", [G, NQ * 128], bf16, kind="ExternalInput").ap()
    iota_d = nc.dram_tensor("iota", [128, 1], f32, kind="ExternalInput").ap()
    ident_d = nc.dram_tensor("ident", [128, 128], bf16, kind="ExternalInput").ap()
    out_d = nc.dram_tensor("out", [BPC, S, D], f32, kind="ExternalOutput").ap()

    MAX = mybir.AluOpType.max

    with tile.TileContext(nc) as tc:
        with (
            tc.tile_pool(name="const", bufs=1) as cpool,
            tc.tile_pool(name="work", bufs=1) as wpool,
            tc.tile_pool(name="rot", bufs=2) as rpool,
            tc.tile_pool(name="pp_planes", bufs=1, space="PSUM") as pp_planes,
            tc.tile_pool(name="pp_patb", bufs=1, space="PSUM") as pp_patb,
            tc.tile_pool(name="pp_tbl", bufs=1, space="PSUM") as pp_tbl,
            tc.tile_pool(name="pp_o", bufs=1, space="PSUM") as pp_o,
        ):
            # ---------------- constants ----------------
            p5a = cpool.tile([128, G], bf16, tag="p5a")
            p5b = cpool.tile([S - 128, G], bf16, tag="p5b")
            nc.sync.dma_start(p5a[:], p5_d[0:128, :])
            nc.sync.dma_start(p5b[:], p5_d[128:S, :])
            rall = cpool.tile([G, NQ * 128], bf16, tag="rall")
            nc.sync.dma_start(rall[:], rall_d[:, :])
            iota = cpool.tile([128, 1], f32, tag="iota")
            nc.sync.dma_start(iota[:], iota_d[:, :])
            ident = cpool.tile([128, 128], bf16, tag="ident")
            nc.sync.dma_start(ident[:], ident_d[:, :])

            # ---------------- weights: load, bf16, transpose ----------------
            ws_sb = cpool.tile([D, D], f32, tag="ws")
            wn_sb = cpool.tile([D, D], f32, tag="wn")
            nc.sync.dma_start(ws_sb[:], ws_d[:, :])
            nc.sync.dma_start(wn_sb[:], wn_d[:, :])
            ws_bf = cpool.tile([D, D], bf16, tag="wsbf")
            wn_bf = cpool.tile([D, D], bf16, tag="wnbf")
            nc.scalar.copy(ws_bf[:], ws_sb[:])
            nc.scalar.copy(wn_bf[:], wn_sb[:])
            wst = cpool.tile([D, D], bf16, tag="wst")
            wnt = cpool.tile([D, D], bf16, tag="wnt")
            for w_bf, w_t in ((ws_bf, wst), (wn_bf, wnt)):
                wt_ps = pp_tbl.tile([128, (NQ // 2) * 128], bf16, tag="tbl")
                nc.tensor.transpose(wt_ps[:, 0:128], w_bf[:], ident[:])
                nc.scalar.copy(w_t[:], wt_ps[:, 0:128])

            # ---------------- x and A loads (all batches) ----------------
            xj0 = wpool.tile([128, BPC * D], f32, tag="xj0")
            xj1 = wpool.tile([S - 128, BPC * D], f32, tag="xj1")
            nc.sync.dma_start(xj0[:].rearrange("j (b d) -> j b d", b=BPC),
                              x_d[:, 0:128, :].rearrange("b s d -> s b d"))
            nc.sync.dma_start(xj1[:].rearrange("j (b d) -> j b d", b=BPC),
                              x_d[:, 128:S, :].rearrange("b s d -> s b d"))
            aj0 = wpool.tile([128, BPC * S], i32, tag="aj0")
            aj1 = wpool.tile([S - 128, BPC * S], i32, tag="aj1")
            nc.sync.dma_start(aj0[:].rearrange("j (b i) -> j b i", b=BPC),
                              a_d[:, 0:128, :].rearrange("b s i -> s b i"))
            nc.sync.dma_start(aj1[:].rearrange("j (b i) -> j b i", b=BPC),
                              a_d[:, 128:S, :].rearrange("b s i -> s b i"))

            xb0 = wpool.tile([128, BPC * D], bf16, tag="xb0")
            xb1 = wpool.tile([S - 128, BPC * D], bf16, tag="xb1")
            nc.scalar.copy(xb0[:], xj0[:])
            nc.scalar.copy(xb1[:], xj1[:])
            ab0 = wpool.tile([128, BPC * S], bf16, tag="ab0")
            ab1 = wpool.tile([S - 128, BPC * S], bf16, tag="ab1")
            nc.scalar.copy(ab0[:], aj0[:])
            nc.scalar.copy(ab1[:], aj1[:])

            xb0v = xb0[:].rearrange("j (b d) -> j b d", b=BPC)
            xb1v = xb1[:].rearrange("j (b d) -> j b d", b=BPC)
            ab0v = ab0[:].rearrange("j (b i) -> j b i", b=BPC)
            ab1v = ab1[:].rearrange("j (b i) -> j b i", b=BPC)

            # x^T for all batches: [d, (b, j)]
            xT = wpool.tile([128, BPC * S], bf16, tag="xT")
            for b in range(BPC):
                xt_ps = pp_tbl.tile([128, (NQ // 2) * 128], bf16, tag="tbl")
                nc.tensor.transpose(xt_ps[:, 0:128], xb0v[:, b, :], ident[:])
                nc.tensor.transpose(xt_ps[:, 128:S], xb1v[:, b, :], ident[0:S - 128, 0:S - 128])
                nc.scalar.copy(xT[:, b * S:(b + 1) * S], xt_ps[:, 0:S])
            xTv = xT[:].rearrange("d (b j) -> d b j", b=BPC)

            for b in range(BPC):
                # ---------------- subset-max lattice (chunk tables) ----------
                # mall[b]: [d, (g: G, s: NS)] bf16
                mall = rpool.tile([128, G * NS], bf16, tag="mall")
                mv = mall[:].rearrange("d (g s) -> d g s", g=G)
                nc.gpsimd.memset(mv[:, :, 0:1], 0.0)           # M[empty] = 0
                xgk = xTv[:, b, :].rearrange("d (g k) -> d g k", g=G)
                for k in range(C):
                    w = 1 << k
                    nc.vector.tensor_tensor(
                        out=mv[:, 0:G, w:2 * w],
                        in0=mv[:, 0:G, 0:w],
                        in1=xgk[:, :, k:k + 1].broadcast_to((128, G, w)),
                        op=MAX,
                    )

                # triple transposes -> [(c, s), d] tables (rows 0..96), to SBUF
                tbl = rpool.tile([128, NQ * 128], bf16, tag="tblsb")
                for half in range(2):
                    tbl_ps = pp_tbl.tile([128, (NQ // 2) * 128], bf16, tag="tbl")
                    for qh in range(NQ // 2):
                        q = half * (NQ // 2) + qh
                        nc.tensor.transpose(
                            tbl_ps[0:96, qh * 128:(qh + 1) * 128],
                            mall[:, q * 96:(q + 1) * 96],
                            ident[:],
                        )
                    nc.scalar.copy(
                        tbl[0:96, half * (NQ // 2) * 128:(half + 1) * (NQ // 2) * 128],
                        tbl_ps[0:96, :],
                    )

                # ---------------- patterns and one-hots ----------------
                patb_ps = pp_patb.tile([128, 1024], f32, tag="patb")
                # Pat[g, i] in f32 PSUM (region of the patb banks)
                nc.tensor.matmul(patb_ps[0:G, 0:S], p5a[:], ab0v[:, b, :],
                                 start=True, stop=False)
                nc.tensor.matmul(patb_ps[0:G, 0:S], p5b[:], ab1v[:, b, :],
                                 start=False, stop=True)
                pat_sb = rpool.tile([G, S], bf16, tag="patsb")
                nc.scalar.copy(pat_sb[:], patb_ps[0:G, 0:S])

                oh = rpool.tile([128, NQ * S], bf16, tag="oh")
                ohv = oh[:].rearrange("p (q i) -> p q i", q=NQ)
                for f, nq in ((0, 4), (1, 4), (2, 2)):
                    for h in range(nq):
                        q = f * 4 + h
                        nc.tensor.matmul(patb_ps[:, h * 256:h * 256 + S],
                                         rall[:, q * 128:(q + 1) * 128],
                                         pat_sb[:], start=True, stop=True)
                    nc.vector.tensor_scalar(
                        out=ohv[:, f * 4:f * 4 + nq, :],
                        in0=patb_ps[:].rearrange("p (h i) -> p h i", h=4)[:, 0:nq, 0:S],
                        scalar1=iota[:], scalar2=None,
                        op0=mybir.AluOpType.is_equal,
                    )

                # ---------------- selection waves + L1 max tree --------------
                NW = 4          # waves of up to 8 chunks (4 PSUM banks)
                # Same-bank matmuls must share the operand row base
                # (tile_position), so order chunks by their base class g % 3;
                # each class has an even count (10) so bank pairs never mix.
                gorder = [g for cls in range(3) for g in range(cls, G, 3)]
                l1 = rpool.tile([128, 15 * S], bf16, tag="l1")
                for w in range(NW):
                    glo = w * 8
                    gn = min(8, G - glo)
                    pl = pp_planes.tile([128, 2048], f32, tag="pl")
                    for j in range(gn):
                        g = gorder[glo + j]
                        q, cc = g // 3, g % 3
                        col = (j // 2) * 512 + (j % 2) * S
                        nc.tensor.matmul(
                            pl[:, col:col + S],
                            tbl[cc * 32:(cc + 1) * 32, q * 128:(q + 1) * 128],
                            ohv[cc * 32:(cc + 1) * 32, q, :],
                            start=True, stop=True,
                        )
                    npair = gn // 2
                    # Verifier allows only ONE PSUM input per DVE op, so fold
                    # each bank's two planes with a tensor_reduce (slot axis
                    # innermost), not a two-input tensor_tensor.
                    red_in = (pl[:].rearrange("p (bk c) -> p bk c", bk=4)
                              [:, 0:npair, 0:2 * S]
                              .rearrange("p bk (sl i) -> p bk i sl", sl=2))
                    nc.vector.tensor_reduce(
                        out=l1[:].rearrange("p (k i) -> p k i", i=S)[:, w * 4:w * 4 + npair, :],
                        in_=red_in, axis=mybir.AxisListType.X, op=MAX,
                    )

                # ---------------- upper tree levels (DVE, bf16 SBUF) ---------
                l1v = l1[:].rearrange("p (k i) -> p k i", i=S)
                l2 = rpool.tile([128, 8 * S], bf16, tag="l2")
                l2v = l2[:].rearrange("p (k i) -> p k i", i=S)
                nc.vector.tensor_tensor(out=l2v[:, 0:7, :],
                                        in0=l1v[:, 0:14:2, :], in1=l1v[:, 1:14:2, :], op=MAX)
                nc.vector.tensor_copy(l2v[:, 7:8, :], l1v[:, 14:15, :])
                l3 = rpool.tile([128, 4 * S], bf16, tag="l3")
                l3v = l3[:].rearrange("p (k i) -> p k i", i=S)
                nc.vector.tensor_tensor(out=l3v[:], in0=l2v[:, 0:8:2, :], in1=l2v[:, 1:8:2, :], op=MAX)
                l4 = rpool.tile([128, 2 * S], bf16, tag="l4")
                l4v = l4[:].rearrange("p (k i) -> p k i", i=S)
                nc.vector.tensor_tensor(out=l4v[:], in0=l3v[:, 0:4:2, :], in1=l3v[:, 1:4:2, :], op=MAX)
                ngh = rpool.tile([128, S], bf16, tag="ngh")
                nc.vector.tensor_tensor(out=ngh[:], in0=l4v[:, 0:1, :], in1=l4v[:, 1:2, :], op=MAX)

                # ---------------- projections ----------------
                o_ps = pp_o.tile([128, 512], f32, tag="o")
                nc.tensor.matmul(o_ps[:, 0:128], xTv[:, b, 0:128], wst[:], start=True, stop=False)
                nc.tensor.matmul(o_ps[:, 0:128], ngh[:, 0:128], wnt[:], start=False, stop=True)
                nc.tensor.matmul(o_ps[0:S - 128, 128:256], xTv[:, b, 128:S], wst[:], start=True, stop=False)
                nc.tensor.matmul(o_ps[0:S - 128, 128:256], ngh[:, 128:S], wnt[:], start=False, stop=True)
                if b == 0:
                    out0 = wpool.tile([128, BPC * D], f32, tag="out0")
                    out1 = wpool.tile([S - 128, BPC * D], f32, tag="out1")
                nc.scalar.copy(out0[:, b * D:(b + 1) * D], o_ps[:, 0:128])
                nc.scalar.copy(out1[:, b * D:(b + 1) * D], o_ps[0:S - 128, 128:256])

            nc.sync.dma_start(out_d[:, 0:128, :].rearrange("b s e -> s b e"),
                              out0[:].rearrange("s (b e) -> s b e", b=BPC))
            nc.sync.dma_start(out_d[:, 128:S, :].rearrange("b s e -> s b e"),
                              out1[:].rearrange("s (b e) -> s b e", b=BPC))

    nc.compile()
    _PROGRAM_CACHE["nc"] = nc
    return nc


def kernel(x, A, W_self, W_neigh, **kwargs):
    x = np.ascontiguousarray(np.asarray(x, dtype=np.float32))
    A = np.ascontiguousarray(np.asarray(A, dtype=np.int32))
    W_self = np.ascontiguousarray(np.asarray(W_self, dtype=np.float32))
    W_neigh = np.ascontiguousarray(np.asarray(W_neigh, dtype=np.float32))

    p5, rall, iota, ident = _consts()
    nc = _build_program()
    in_maps = [
        {
            "x": x[c * BPC:(c + 1) * BPC],
            "A": A[c * BPC:(c + 1) * BPC],
            "ws": W_self,
            "wn": W_neigh,
            "p5": p5,
            "rall": rall,
            "iota": iota,
            "ident": ident,
        }
        for c in range(NCORES)
    ]
    res = run_bass_kernel_spmd(nc, in_maps, core_ids=list(range(NCORES)), **kwargs)
    out = np.concatenate([res.results[c]["out"] for c in range(NCORES)], axis=0)
    return np.ascontiguousarray(out.astype(np.float32))
